# revision 23
# baseline (speedup 1.0000x reference)
"""Trainium2 Bass kernel for nn_MemoryNetwork (scatter_memory).

Computation (reference, per batch row b):
    f = feature / ||feature||                       [B, 768]
    topic = f @ W_topic.T ; dom = f @ W_domain.T    [B, 256]
    att   = softmax_m(TAU * topic . memory[d,m])    [B, 9, 10]
    sep   = sum_m att * memory[d,m]                 [B, 9, 256]
    out   = softmax_d(TAU * sep . dom)              [B, 1, 9]

Reformulation: fold the tiny memory banks into the projections on host:
    S = (TAU f/||f||) @ (mem_flat@W_topic).T   [B, 90]   (softmax_m logits)
    T = (TAU f/||f||) @ (mem_flat@W_domain).T  [B, 90]
    ex    = exp(S - SHIFT)
    sums  = sum_m ex ; wsum = sum_m ex*T       (device outputs, [9, B])
    out   = softmax_d(wsum/sums)               (host, exact fp64)

Device layout: R = [S|T]-projection matrices are the STATIONARY operand
([128 x 90] per 128-row contraction chunk, reused across the whole
batch); the batch streams as the moving operand in 512-column blocks.
This amortizes PE weight loads -- a feature-stationary scheme pays a
~104ns LDWEIGHTS for every ~75ns of streaming.

Precision: the main product runs as fp16 x fp16 (exact products, fp32
PSUM accumulate): f' = a16 + b, R = C16 + D with a16/C16 fp16.
a16@C16 gives the bulk; the two correction terms run as fp8e4m3
DoubleRow matmuls at 1/4 the cycles of an fp16 pair:
    b8@C8   (b scaled 2^6, C16 scaled 2^-6)
    a8@D8   (f' scaled 2^-7, D scaled 2^7)
Dropped b@D term is O(2^-24). The m-group reductions are selector
matmuls with f32r moving (1 cyc/row, tf32-truncated mantissas -- the
dominant remaining error, ~4e-3 rel end to end).

All constants are host-prepacked to [128, k*cols] partition-major so
their DMAs are single contiguous 2D transfers (a rearranging DMA costs
~1.9us of descriptor generation and serializes the queue -- that idled
the PE >3.4us and re-throttled HAM to 1.2 GHz in an earlier version).

Sharding: data-parallel over B across 8 cores (4096 rows each).
"""

import sys

sys.path.insert(0, "/opt/trn_rl_repo")

import numpy as np

B, IN, E, D, M = 32768, 768, 256, 9, 10
NCORES = 8
BC = B // NCORES   # rows per core (4096)
P = 128            # partition tile
KC = IN // P       # contraction chunks (6)
NBC = 512          # batch block columns (one PSUM bank of fp32)
NB = BC // NBC     # blocks per core (8)
DM = D * M         # 90
DMP = 96           # DoubleRow stationary pad (sub-k stride % 16 == 0)
TAU = 32.0
SHIFT = 50.0
SB = 6             # b-plane scale exponent (b*2^SB, C*2^-SB)
GA = 7             # a8/D8 scale exponent (f*2^-GA, D*2^GA)

_CACHE: dict = {}


def _build_nc(repeat=1):
    from contextlib import ExitStack

    import concourse.bacc as bacc
    import concourse.tile as tile
    from concourse import mybir

    F32 = mybir.dt.float32
    F32R = mybir.dt.float32r
    F16 = mybir.dt.float16
    F8 = mybir.dt.float8e4
    AF = mybir.ActivationFunctionType
    DR = mybir.MatmulPerfMode.DoubleRow

    nc = bacc.Bacc(trn_type="TRN2")
    # feature planes host-packed [P, NB * KC*NBC]: one contiguous segment
    # per partition per (block) -> single-descriptor DMAs
    a16 = nc.dram_tensor("a16", [P, NB * KC * NBC], F16, kind="ExternalInput")
    b8 = nc.dram_tensor("b8", [P, NB * KC * NBC], F8, kind="ExternalInput")
    a8 = nc.dram_tensor("a8", [P, NB * KC * NBC], F8, kind="ExternalInput")
    # consts prepacked [128, KC*cols] partition-major
    rsh = nc.dram_tensor("rsh", [P, KC * DM], F16, kind="ExternalInput")
    rth = nc.dram_tensor("rth", [P, KC * DM], F16, kind="ExternalInput")
    c8s = nc.dram_tensor("c8s", [P, KC * DMP], F8, kind="ExternalInput")
    c8t = nc.dram_tensor("c8t", [P, KC * DMP], F8, kind="ExternalInput")
    d8s = nc.dram_tensor("d8s", [P, KC * DMP], F8, kind="ExternalInput")
    d8t = nc.dram_tensor("d8t", [P, KC * DMP], F8, kind="ExternalInput")
    sel = nc.dram_tensor("sel", [DM, D], F32R, kind="ExternalInput")
    sw_d = nc.dram_tensor("sw", [D, 2 * BC], F32, kind="ExternalOutput")

    with tile.TileContext(nc) as tc, ExitStack() as ctx:
        const = ctx.enter_context(tc.tile_pool(name="const", bufs=1))
        fpool = ctx.enter_context(tc.tile_pool(name="fts", bufs=2))
        spool = ctx.enter_context(tc.tile_pool(name="small", bufs=2))
        mm_ps = ctx.enter_context(tc.tile_pool(name="mmps", bufs=2, space="PSUM"))
        red_ps = ctx.enter_context(tc.tile_pool(name="redps", bufs=2, space="PSUM"))

        def ld_const(name, dram, cols, dtype, eng):
            t = const.tile([P, KC, cols], dtype, name=name)
            eng.dma_start(t[:], dram[:, :].rearrange("p (k j) -> p k j", k=KC))
            return t

        # scalar-queue consts lead (rsh feeds the very first matmul);
        # sync-queue consts are issued after block 0's a16 chunks below
        rsh_sb = ld_const("rsh_sb", rsh, DM, F16, nc.scalar)
        c8s_sb = ld_const("c8s_sb", c8s, DMP, F8, nc.scalar)
        d8s_sb = ld_const("d8s_sb", d8s, DMP, F8, nc.scalar)
        sel_sb = const.tile([DM, D], F32R)
        nc.scalar.dma_start(sel_sb[:], sel[:, :])
        bias_sb = const.tile([P, 1], F32)
        nc.gpsimd.memset(bias_sb[:], -SHIFT)

        a16_v = a16[:, :].rearrange("p (n k b) -> p n k b", n=NB, k=KC)
        b8_v = b8[:, :].rearrange("p (n k b) -> p n k b", n=NB, k=KC)
        a8_v = a8[:, :].rearrange("p (n k b) -> p n k b", n=NB, k=KC)

        # PE warmup during the DMA ramp: dense zero matmuls flip the HAM
        # clock gate (4096-cycle activity window) to 8/8 before real work
        warm_w = const.tile([P, DM], F16)
        nc.gpsimd.memset(warm_w[:], 0.0)
        warm_m = const.tile([P, NBC], F16)
        nc.gpsimd.memset(warm_m[:], 0.0)
        ps_warm = mm_ps.tile([DM, NBC], F32, tag="ps_s")
        for _ in range(10):
            nc.tensor.matmul(ps_warm[:], warm_w[:], warm_m[:], start=True, stop=True)

        sw_v = sw_d[:, :].rearrange("d (x b) -> d x b", x=2)
        rth_sb = c8t_sb = d8t_sb = None
        pend = None  # deferred tail of the previous block

        def tail_back(ex, prod, cols, halves):
            sw_sb = spool.tile([D, 2, NBC], F32, tag="sw", name="sw_sb")
            for h in halves:
                ps_sums = red_ps.tile([D, NBC], F32, tag="sums", name="ps_sums")
                ps_wsum = red_ps.tile([D, NBC], F32, tag="wsum", name="ps_wsum")
                nc.tensor.matmul(
                    ps_sums[:, h], sel_sb[:], ex[:, h], start=True, stop=True
                )
                nc.tensor.matmul(
                    ps_wsum[:, h], sel_sb[:], prod[:, h], start=True, stop=True
                )
                nc.vector.tensor_copy(sw_sb[:, 0, h], ps_sums[:, h])
                nc.vector.tensor_copy(sw_sb[:, 1, h], ps_wsum[:, h])
                nc.scalar.dma_start(
                    sw_v[:, :, cols.start + h.start : cols.start + h.stop],
                    sw_sb[:, :, h],
                )

        for it in range(NB * repeat):
            blk = it % NB
            cols = slice(blk * NBC, (blk + 1) * NBC)

            a16_sb = fpool.tile([P, KC, NBC], F16, tag="a16")
            b8_sb = fpool.tile([P, KC, NBC], F8, tag="b8")
            a8_sb = fpool.tile([P, KC, NBC], F8, tag="a8")
            if it == 0:
                # block 0 rides one queue in exact need-order: per-chunk a16
                # first (first matmul starts on chunk 0), then rth for the
                # T-mains, then the correction planes and fp8 stationaries
                for k in range(KC):
                    nc.sync.dma_start(a16_sb[:, k, :], a16_v[:, blk, k])
                rth_sb = ld_const("rth_sb", rth, DM, F16, nc.sync)
                nc.sync.dma_start(b8_sb[:], b8_v[:, blk])
                nc.sync.dma_start(a8_sb[:], a8_v[:, blk])
                c8t_sb = ld_const("c8t_sb", c8t, DMP, F8, nc.sync)
                d8t_sb = ld_const("d8t_sb", d8t, DMP, F8, nc.sync)
            else:
                nc.sync.dma_start(a16_sb[:], a16_v[:, blk])
                nc.gpsimd.dma_start(b8_sb[:], b8_v[:, blk])
                nc.scalar.dma_start(a8_sb[:], a8_v[:, blk])

            ps_s = mm_ps.tile([DM, NBC], F32, tag="ps_s")
            ps_t = mm_ps.tile([DM, NBC], F32, tag="ps_t")
            for ps, r16, c8, d8 in (
                (ps_s, rsh_sb, c8s_sb, d8s_sb),
                (ps_t, rth_sb, c8t_sb, d8t_sb),
            ):
                for k in range(KC):
                    nc.tensor.matmul(
                        ps[:], r16[:, k, :], a16_sb[:, k, :],
                        start=(k == 0), stop=False,
                    )
                for kk in range(0, KC, 2):
                    nc.tensor.matmul(
                        ps[:], c8[:, kk : kk + 2, 0:DM], b8_sb[:, kk : kk + 2, :],
                        start=False, stop=False, perf_mode=DR,
                    )
                for kk in range(0, KC, 2):
                    nc.tensor.matmul(
                        ps[:], d8[:, kk : kk + 2, 0:DM], a8_sb[:, kk : kk + 2, :],
                        start=False, stop=(kk == KC - 2), perf_mode=DR,
                    )

            # previous block's reductions slot in here: their ex/prod are
            # long done, so the PE never stalls on the ACT->DVE chain
            if pend is not None:
                tail_back(*pend)

            ex = spool.tile([DM, NBC], F32R, tag="ex")
            nc.scalar.activation(ex[:], ps_s[:], AF.Exp, bias=bias_sb[0:DM])
            prod = spool.tile([DM, NBC], F32R, tag="prod")
            nc.vector.tensor_mul(prod[:], ex[:], ps_t[:])
            pend = (ex, prod, cols, ((slice(0, NBC),)))

        # drain the last block with a split tail
        ex, prod, cols, _ = pend
        tail_back(ex, prod, cols, (slice(0, NBC // 2), slice(NBC // 2, NBC)))

    nc.finalize()
    return nc


def _get_nc():
    if "nc" not in _CACHE:
        _CACHE["nc"] = _build_nc()
    return _CACHE["nc"]


def _host_prep(feature, W_topic, W_domain, memory):
    """Fold memory banks into projections; fp16+fp8 split planes of the
    TAU-scaled row-normalized features, transposed per core."""
    import ml_dtypes

    E4 = ml_dtypes.float8_e4m3

    mem_flat = memory.reshape(D * M, E).astype(np.float64)
    Pm = mem_flat @ W_topic.astype(np.float64)  # [90, 768]
    Qm = mem_flat @ W_domain.astype(np.float64)

    def pack(A):  # [768, cols] -> [128, KC*cols] partition-major
        cols = A.shape[1]
        return np.ascontiguousarray(
            A.reshape(KC, P, cols).transpose(1, 0, 2).reshape(P, KC * cols)
        )

    def r_planes(R):
        C16 = R.astype(np.float32).astype(np.float16)  # [90, 768]
        c8 = (C16.astype(np.float64) * 2.0**-SB).astype(np.float32).astype(E4)
        d8 = ((R - C16.astype(np.float64)) * 2.0**GA).astype(np.float32).astype(E4)
        r16 = pack(np.ascontiguousarray(C16.T))
        padc = np.zeros((IN, DMP), E4)
        padc[:, :DM] = c8.T
        padd = np.zeros((IN, DMP), E4)
        padd[:, :DM] = d8.T
        return r16, pack(padc), pack(padd)

    rsh, c8s, d8s = r_planes(Pm)
    rth, c8t, d8t = r_planes(Qm)
    sel = np.zeros((DM, D), np.float32)
    sel[np.arange(DM), np.arange(DM) // M] = 1.0

    f = np.asarray(feature, dtype=np.float32)
    norm = np.sqrt((f.astype(np.float64) ** 2).sum(axis=1))
    fn = f.astype(np.float64) * (TAU / norm)[:, None]
    a16f = fn.astype(np.float16)                      # [B, 768]
    b8f = ((fn - a16f.astype(np.float64)) * 2.0**SB).astype(np.float32).astype(E4)
    a8f = (fn * 2.0**-GA).astype(np.float32).astype(E4)

    def pack_feat(A):  # [768, BC] -> [P, NB * KC*NBC] block-contiguous
        return np.ascontiguousarray(
            A.reshape(KC, P, NB, NBC)
            .transpose(1, 2, 0, 3)
            .reshape(P, NB * KC * NBC)
        )

    per_core = []
    for c in range(NCORES):
        rows = slice(c * BC, (c + 1) * BC)
        per_core.append(
            {
                "a16": pack_feat(a16f[rows].T),
                "b8": pack_feat(b8f[rows].T),
                "a8": pack_feat(a8f[rows].T),
                "rsh": rsh, "rth": rth,
                "c8s": c8s, "c8t": c8t, "d8s": d8s, "d8t": d8t,
                "sel": sel,
            }
        )
    return per_core


def kernel(feature, category, W_topic, W_domain, memory):
    from concourse.bass_utils import run_bass_kernel_spmd

    in_maps = _host_prep(
        feature, np.asarray(W_topic), np.asarray(W_domain), np.asarray(memory)
    )
    nc = _get_nc()
    res = run_bass_kernel_spmd(nc, in_maps, core_ids=list(range(NCORES)))
    outs = []
    for c in range(NCORES):
        sw = res.results[c]["sw"].astype(np.float64)  # [9, 2*BC]
        datt = sw[:, BC:] / sw[:, :BC]
        e2 = np.exp(datt - datt.max(axis=0, keepdims=True))
        outs.append((e2 / e2.sum(axis=0, keepdims=True)).T)
    full = np.concatenate(outs, axis=0)  # [B, 9]
    return full[:, None, :].astype(np.float32)


# revision 25
# speedup vs baseline: 1.1414x; 1.1414x over previous
"""Trainium2 Bass kernel for nn_MemoryNetwork (scatter_memory).

Computation (reference, per batch row b):
    f = feature / ||feature||                       [B, 768]
    topic = f @ W_topic.T ; dom = f @ W_domain.T    [B, 256]
    att   = softmax_m(TAU * topic . memory[d,m])    [B, 9, 10]
    sep   = sum_m att * memory[d,m]                 [B, 9, 256]
    out   = softmax_d(TAU * sep . dom)              [B, 1, 9]

Reformulation: fold the tiny memory banks into the projections on host:
    S = (TAU f/||f||) @ (mem_flat@W_topic).T   [B, 90]   (softmax_m logits)
    T = (TAU f/||f||) @ (mem_flat@W_domain).T  [B, 90]
    ex    = exp(S - SHIFT)
    sums  = sum_m ex ; wsum = sum_m ex*T       (device outputs, [9, B])
    out   = softmax_d(wsum/sums)               (host, exact fp64)

Device layout: R = [S|T]-projection matrices are the STATIONARY operand
([128 x 90] per 128-row contraction chunk, reused across the whole
batch); the batch streams as the moving operand in 512-column blocks.
This amortizes PE weight loads -- a feature-stationary scheme pays a
~104ns LDWEIGHTS for every ~75ns of streaming.

Precision: the main product runs as fp16 x fp16 (exact products, fp32
PSUM accumulate): f' = a16 + b, R = C16 + D with a16/C16 fp16.
a16@C16 gives the bulk; the two correction terms run as fp8e4m3
DoubleRow matmuls at 1/4 the cycles of an fp16 pair:
    b8@C8   (b scaled 2^6, C16 scaled 2^-6)
    a8@D8   (f' scaled 2^-7, D scaled 2^7)
Dropped b@D term is O(2^-24). The m-group reductions are selector
matmuls with f32r moving (1 cyc/row, tf32-truncated mantissas -- the
dominant remaining error, ~4e-3 rel end to end).

All constants are host-prepacked to [128, k*cols] partition-major so
their DMAs are single contiguous 2D transfers (a rearranging DMA costs
~1.9us of descriptor generation and serializes the queue -- that idled
the PE >3.4us and re-throttled HAM to 1.2 GHz in an earlier version).

Sharding: data-parallel over B across 8 cores (4096 rows each).
"""

import sys

sys.path.insert(0, "/opt/trn_rl_repo")

import numpy as np

B, IN, E, D, M = 32768, 768, 256, 9, 10
NCORES = 8
BC = B // NCORES   # rows per core (4096)
P = 128            # partition tile
KC = IN // P       # contraction chunks (6)
NBC = 512          # batch block columns (one PSUM bank of fp32)
NB = BC // NBC     # blocks per core (8)
DM = D * M         # 90
DMP = 96           # DoubleRow stationary pad (sub-k stride % 16 == 0)
TAU = 32.0
SHIFT = 50.0
SB = 6             # b-plane scale exponent (b*2^SB, C*2^-SB)
GA = 7             # a8/D8 scale exponent (f*2^-GA, D*2^GA)

_CACHE: dict = {}


def _build_nc(repeat=1):
    from contextlib import ExitStack

    import concourse.bacc as bacc
    import concourse.tile as tile
    from concourse import mybir

    F32 = mybir.dt.float32
    F32R = mybir.dt.float32r
    F16 = mybir.dt.float16
    F8 = mybir.dt.float8e4
    AF = mybir.ActivationFunctionType
    DR = mybir.MatmulPerfMode.DoubleRow

    nc = bacc.Bacc(trn_type="TRN2")
    # feature planes host-packed [P, NB * KC*NBC]: one contiguous segment
    # per partition per (block) -> single-descriptor DMAs
    a16 = nc.dram_tensor("a16", [P, NB * KC * NBC], F16, kind="ExternalInput")
    b8 = nc.dram_tensor("b8", [P, NB * KC * NBC], F8, kind="ExternalInput")
    a8 = nc.dram_tensor("a8", [P, NB * KC * NBC], F8, kind="ExternalInput")
    # consts prepacked [128, KC*cols] partition-major
    rsh = nc.dram_tensor("rsh", [P, KC * DM], F16, kind="ExternalInput")
    rth = nc.dram_tensor("rth", [P, KC * DM], F16, kind="ExternalInput")
    c8s = nc.dram_tensor("c8s", [P, KC * DMP], F8, kind="ExternalInput")
    c8t = nc.dram_tensor("c8t", [P, KC * DMP], F8, kind="ExternalInput")
    d8s = nc.dram_tensor("d8s", [P, KC * DMP], F8, kind="ExternalInput")
    d8t = nc.dram_tensor("d8t", [P, KC * DMP], F8, kind="ExternalInput")
    sel = nc.dram_tensor("sel", [DM, D], F32R, kind="ExternalInput")
    sw_d = nc.dram_tensor("sw", [D, 2 * BC], F32, kind="ExternalOutput")

    with tile.TileContext(nc) as tc, ExitStack() as ctx:
        const = ctx.enter_context(tc.tile_pool(name="const", bufs=1))
        fpool = ctx.enter_context(tc.tile_pool(name="fts", bufs=2))
        spool = ctx.enter_context(tc.tile_pool(name="small", bufs=2))
        mm_ps = ctx.enter_context(tc.tile_pool(name="mmps", bufs=2, space="PSUM"))
        red_ps = ctx.enter_context(tc.tile_pool(name="redps", bufs=2, space="PSUM"))

        def ld_const(name, dram, cols, dtype, eng):
            t = const.tile([P, KC, cols], dtype, name=name)
            eng.dma_start(t[:], dram[:, :].rearrange("p (k j) -> p k j", k=KC))
            return t

        # scalar-queue consts lead (rsh feeds the very first matmul);
        # sync-queue consts are issued after block 0's a16 chunks below
        rsh_sb = ld_const("rsh_sb", rsh, DM, F16, nc.scalar)
        c8s_sb = ld_const("c8s_sb", c8s, DMP, F8, nc.scalar)
        d8s_sb = ld_const("d8s_sb", d8s, DMP, F8, nc.scalar)
        sel_sb = const.tile([DM, D], F32R)
        nc.scalar.dma_start(sel_sb[:], sel[:, :])
        bias_sb = const.tile([P, 1], F32)
        nc.gpsimd.memset(bias_sb[:], -SHIFT)

        a16_v = a16[:, :].rearrange("p (n k b) -> p n k b", n=NB, k=KC)
        b8_v = b8[:, :].rearrange("p (n k b) -> p n k b", n=NB, k=KC)
        a8_v = a8[:, :].rearrange("p (n k b) -> p n k b", n=NB, k=KC)

        # PE warmup during the DMA ramp: dense zero matmuls flip the HAM
        # clock gate (4096-cycle activity window) to 8/8 before real work
        warm_w = const.tile([P, DM], F16)
        nc.gpsimd.memset(warm_w[:], 0.0)
        warm_m = const.tile([P, NBC], F16)
        nc.gpsimd.memset(warm_m[:], 0.0)
        ps_warm = mm_ps.tile([DM, NBC], F32, tag="ps_s")
        for _ in range(8):
            nc.tensor.matmul(ps_warm[:], warm_w[:], warm_m[:], start=True, stop=True)

        sw_v = sw_d[:, :].rearrange("d (x b) -> d x b", x=2)
        rth_sb = c8t_sb = d8t_sb = None
        pend = None  # deferred tail of the previous block

        def tail_back(ex, prod, cols, halves):
            sw_sb = spool.tile([D, 2, NBC], F32, tag="sw", name="sw_sb")
            for h in halves:
                ps_sums = red_ps.tile([D, NBC], F32, tag="sums", name="ps_sums")
                ps_wsum = red_ps.tile([D, NBC], F32, tag="wsum", name="ps_wsum")
                nc.tensor.matmul(
                    ps_sums[:, h], sel_sb[:], ex[:, h], start=True, stop=True
                )
                nc.tensor.matmul(
                    ps_wsum[:, h], sel_sb[:], prod[:, h], start=True, stop=True
                )
                nc.vector.tensor_copy(sw_sb[:, 0, h], ps_sums[:, h])
                nc.vector.tensor_copy(sw_sb[:, 1, h], ps_wsum[:, h])
                nc.scalar.dma_start(
                    sw_v[:, :, cols.start + h.start : cols.start + h.stop],
                    sw_sb[:, :, h],
                )

        for it in range(NB * repeat):
            blk = it % NB
            cols = slice(blk * NBC, (blk + 1) * NBC)

            a16_sb = fpool.tile([P, KC, NBC], F16, tag="a16")
            b8_sb = fpool.tile([P, KC, NBC], F8, tag="b8")
            a8_sb = fpool.tile([P, KC, NBC], F8, tag="a8")
            if it == 0:
                # block 0 rides one queue in exact need-order: per-chunk a16
                # first (first matmul starts on chunk 0), then rth for the
                # T-mains, then the correction planes and fp8 stationaries
                for k in range(KC):
                    nc.sync.dma_start(a16_sb[:, k, :], a16_v[:, blk, k])
                rth_sb = ld_const("rth_sb", rth, DM, F16, nc.sync)
                nc.sync.dma_start(b8_sb[:], b8_v[:, blk])
                nc.sync.dma_start(a8_sb[:], a8_v[:, blk])
                c8t_sb = ld_const("c8t_sb", c8t, DMP, F8, nc.sync)
                d8t_sb = ld_const("d8t_sb", d8t, DMP, F8, nc.sync)
            else:
                nc.sync.dma_start(a16_sb[:], a16_v[:, blk])
                nc.gpsimd.dma_start(b8_sb[:], b8_v[:, blk])
                nc.scalar.dma_start(a8_sb[:], a8_v[:, blk])

            # all fp16 mains first, then all DoubleRow corrections: one
            # fp16->DR transition per block instead of two (the first DR
            # matmul's 360-col weight load doesn't hide, ~190ns each).
            # Interleaving the two PSUM accumulation groups needs
            # skip_group_check; hardware has_written bits are per element.
            ps_s = mm_ps.tile([DM, NBC], F32, tag="ps_s")
            ps_t = mm_ps.tile([DM, NBC], F32, tag="ps_t")
            for ps, r16 in ((ps_s, rsh_sb), (ps_t, rth_sb)):
                for k in range(KC):
                    nc.tensor.matmul(
                        ps[:], r16[:, k, :], a16_sb[:, k, :],
                        start=(k == 0), stop=False, skip_group_check=True,
                    )
            for ps, c8, d8 in ((ps_s, c8s_sb, d8s_sb), (ps_t, c8t_sb, d8t_sb)):
                for kk in range(0, KC, 2):
                    nc.tensor.matmul(
                        ps[:], c8[:, kk : kk + 2, 0:DM], b8_sb[:, kk : kk + 2, :],
                        start=False, stop=False, perf_mode=DR,
                        skip_group_check=True,
                    )
                for kk in range(0, KC, 2):
                    nc.tensor.matmul(
                        ps[:], d8[:, kk : kk + 2, 0:DM], a8_sb[:, kk : kk + 2, :],
                        start=False, stop=(kk == KC - 2), perf_mode=DR,
                        skip_group_check=True,
                    )

            # previous block's reductions slot in here: their ex/prod are
            # long done, so the PE never stalls on the ACT->DVE chain
            if pend is not None:
                tail_back(*pend)

            ex = spool.tile([DM, NBC], F32R, tag="ex")
            nc.scalar.activation(ex[:], ps_s[:], AF.Exp, bias=bias_sb[0:DM])
            prod = spool.tile([DM, NBC], F32R, tag="prod")
            nc.vector.tensor_mul(prod[:], ex[:], ps_t[:])
            pend = (ex, prod, cols, ((slice(0, NBC),)))

        # drain the last block with a split tail
        ex, prod, cols, _ = pend
        tail_back(ex, prod, cols, (slice(0, NBC // 2), slice(NBC // 2, NBC)))

    nc.finalize()
    return nc


def _get_nc():
    if "nc" not in _CACHE:
        _CACHE["nc"] = _build_nc()
    return _CACHE["nc"]


def _host_prep(feature, W_topic, W_domain, memory):
    """Fold memory banks into projections; fp16+fp8 split planes of the
    TAU-scaled row-normalized features, transposed per core."""
    import ml_dtypes

    E4 = ml_dtypes.float8_e4m3

    mem_flat = memory.reshape(D * M, E).astype(np.float64)
    Pm = mem_flat @ W_topic.astype(np.float64)  # [90, 768]
    Qm = mem_flat @ W_domain.astype(np.float64)

    def pack(A):  # [768, cols] -> [128, KC*cols] partition-major
        cols = A.shape[1]
        return np.ascontiguousarray(
            A.reshape(KC, P, cols).transpose(1, 0, 2).reshape(P, KC * cols)
        )

    def r_planes(R):
        C16 = R.astype(np.float32).astype(np.float16)  # [90, 768]
        c8 = (C16.astype(np.float64) * 2.0**-SB).astype(np.float32).astype(E4)
        d8 = ((R - C16.astype(np.float64)) * 2.0**GA).astype(np.float32).astype(E4)
        r16 = pack(np.ascontiguousarray(C16.T))
        padc = np.zeros((IN, DMP), E4)
        padc[:, :DM] = c8.T
        padd = np.zeros((IN, DMP), E4)
        padd[:, :DM] = d8.T
        return r16, pack(padc), pack(padd)

    rsh, c8s, d8s = r_planes(Pm)
    rth, c8t, d8t = r_planes(Qm)
    sel = np.zeros((DM, D), np.float32)
    sel[np.arange(DM), np.arange(DM) // M] = 1.0

    f = np.asarray(feature, dtype=np.float32)
    norm = np.sqrt((f.astype(np.float64) ** 2).sum(axis=1))
    fn = f.astype(np.float64) * (TAU / norm)[:, None]
    a16f = fn.astype(np.float16)                      # [B, 768]
    b8f = ((fn - a16f.astype(np.float64)) * 2.0**SB).astype(np.float32).astype(E4)
    a8f = (fn * 2.0**-GA).astype(np.float32).astype(E4)

    def pack_feat(A):  # [768, BC] -> [P, NB * KC*NBC] block-contiguous
        return np.ascontiguousarray(
            A.reshape(KC, P, NB, NBC)
            .transpose(1, 2, 0, 3)
            .reshape(P, NB * KC * NBC)
        )

    per_core = []
    for c in range(NCORES):
        rows = slice(c * BC, (c + 1) * BC)
        per_core.append(
            {
                "a16": pack_feat(a16f[rows].T),
                "b8": pack_feat(b8f[rows].T),
                "a8": pack_feat(a8f[rows].T),
                "rsh": rsh, "rth": rth,
                "c8s": c8s, "c8t": c8t, "d8s": d8s, "d8t": d8t,
                "sel": sel,
            }
        )
    return per_core


def kernel(feature, category, W_topic, W_domain, memory):
    from concourse.bass_utils import run_bass_kernel_spmd

    in_maps = _host_prep(
        feature, np.asarray(W_topic), np.asarray(W_domain), np.asarray(memory)
    )
    nc = _get_nc()
    res = run_bass_kernel_spmd(nc, in_maps, core_ids=list(range(NCORES)))
    outs = []
    for c in range(NCORES):
        sw = res.results[c]["sw"].astype(np.float64)  # [9, 2*BC]
        datt = sw[:, BC:] / sw[:, :BC]
        e2 = np.exp(datt - datt.max(axis=0, keepdims=True))
        outs.append((e2 / e2.sum(axis=0, keepdims=True)).T)
    full = np.concatenate(outs, axis=0)  # [B, 9]
    return full[:, None, :].astype(np.float32)


# revision 26
# speedup vs baseline: 1.1469x; 1.0049x over previous
"""Trainium2 Bass kernel for nn_MemoryNetwork (scatter_memory).

Computation (reference, per batch row b):
    f = feature / ||feature||                       [B, 768]
    topic = f @ W_topic.T ; dom = f @ W_domain.T    [B, 256]
    att   = softmax_m(TAU * topic . memory[d,m])    [B, 9, 10]
    sep   = sum_m att * memory[d,m]                 [B, 9, 256]
    out   = softmax_d(TAU * sep . dom)              [B, 1, 9]

Reformulation: fold the tiny memory banks into the projections on host:
    S = (TAU f/||f||) @ (mem_flat@W_topic).T   [B, 90]   (softmax_m logits)
    T = (TAU f/||f||) @ (mem_flat@W_domain).T  [B, 90]
    ex    = exp(S - SHIFT)
    sums  = sum_m ex ; wsum = sum_m ex*T       (device outputs, [9, B])
    out   = softmax_d(wsum/sums)               (host, exact fp64)

Device layout: R = [S|T]-projection matrices are the STATIONARY operand
([128 x 90] per 128-row contraction chunk, reused across the whole
batch); the batch streams as the moving operand in 512-column blocks.
This amortizes PE weight loads -- a feature-stationary scheme pays a
~104ns LDWEIGHTS for every ~75ns of streaming.

Precision: the main product runs as fp16 x fp16 (exact products, fp32
PSUM accumulate): f' = a16 + b, R = C16 + D with a16/C16 fp16.
a16@C16 gives the bulk; the two correction terms run as fp8e4m3
DoubleRow matmuls at 1/4 the cycles of an fp16 pair:
    b8@C8   (b scaled 2^6, C16 scaled 2^-6)
    a8@D8   (f' scaled 2^-7, D scaled 2^7)
Dropped b@D term is O(2^-24). The m-group reductions are selector
matmuls with f32r moving (1 cyc/row, tf32-truncated mantissas -- the
dominant remaining error, ~4e-3 rel end to end).

All constants are host-prepacked to [128, k*cols] partition-major so
their DMAs are single contiguous 2D transfers (a rearranging DMA costs
~1.9us of descriptor generation and serializes the queue -- that idled
the PE >3.4us and re-throttled HAM to 1.2 GHz in an earlier version).

Sharding: data-parallel over B across 8 cores (4096 rows each).
"""

import sys

sys.path.insert(0, "/opt/trn_rl_repo")

import numpy as np

B, IN, E, D, M = 32768, 768, 256, 9, 10
NCORES = 8
BC = B // NCORES   # rows per core (4096)
P = 128            # partition tile
KC = IN // P       # contraction chunks (6)
NBC = 512          # batch block columns (one PSUM bank of fp32)
NB = BC // NBC     # blocks per core (8)
DM = D * M         # 90
DMP = 96           # DoubleRow stationary pad (sub-k stride % 16 == 0)
TAU = 32.0
SHIFT = 50.0
SB = 6             # b-plane scale exponent (b*2^SB, C*2^-SB)
GA = 7             # a8/D8 scale exponent (f*2^-GA, D*2^GA)

_CACHE: dict = {}


def _build_nc(repeat=1):
    from contextlib import ExitStack

    import concourse.bacc as bacc
    import concourse.tile as tile
    from concourse import mybir

    F32 = mybir.dt.float32
    F32R = mybir.dt.float32r
    F16 = mybir.dt.float16
    F8 = mybir.dt.float8e4
    AF = mybir.ActivationFunctionType
    DR = mybir.MatmulPerfMode.DoubleRow

    nc = bacc.Bacc(trn_type="TRN2")
    # feature planes host-packed [P, NB * KC*NBC]: one contiguous segment
    # per partition per (block) -> single-descriptor DMAs
    a16 = nc.dram_tensor("a16", [P, NB * KC * NBC], F16, kind="ExternalInput")
    b8 = nc.dram_tensor("b8", [P, NB * KC * NBC], F8, kind="ExternalInput")
    a8 = nc.dram_tensor("a8", [P, NB * KC * NBC], F8, kind="ExternalInput")
    # consts prepacked [128, KC*cols] partition-major
    rsh = nc.dram_tensor("rsh", [P, KC * DM], F16, kind="ExternalInput")
    rth = nc.dram_tensor("rth", [P, KC * DM], F16, kind="ExternalInput")
    c8s = nc.dram_tensor("c8s", [P, KC * DMP], F8, kind="ExternalInput")
    c8t = nc.dram_tensor("c8t", [P, KC * DMP], F8, kind="ExternalInput")
    d8s = nc.dram_tensor("d8s", [P, KC * DMP], F8, kind="ExternalInput")
    d8t = nc.dram_tensor("d8t", [P, KC * DMP], F8, kind="ExternalInput")
    sel = nc.dram_tensor("sel", [DM, D], F32R, kind="ExternalInput")
    sw_d = nc.dram_tensor("sw", [D, 2 * BC], F32, kind="ExternalOutput")

    with tile.TileContext(nc) as tc, ExitStack() as ctx:
        const = ctx.enter_context(tc.tile_pool(name="const", bufs=1))
        fpool = ctx.enter_context(tc.tile_pool(name="fts", bufs=2))
        spool = ctx.enter_context(tc.tile_pool(name="small", bufs=2))
        mm_ps = ctx.enter_context(tc.tile_pool(name="mmps", bufs=2, space="PSUM"))
        red_ps = ctx.enter_context(tc.tile_pool(name="redps", bufs=2, space="PSUM"))

        def ld_const(name, dram, cols, dtype, eng):
            t = const.tile([P, KC, cols], dtype, name=name)
            eng.dma_start(t[:], dram[:, :].rearrange("p (k j) -> p k j", k=KC))
            return t

        # scalar-queue consts lead (rsh feeds the very first matmul);
        # sync-queue consts are issued after block 0's a16 chunks below
        rsh_sb = ld_const("rsh_sb", rsh, DM, F16, nc.scalar)
        c8s_sb = ld_const("c8s_sb", c8s, DMP, F8, nc.scalar)
        d8s_sb = ld_const("d8s_sb", d8s, DMP, F8, nc.scalar)
        sel_sb = const.tile([DM, D], F32R)
        nc.scalar.dma_start(sel_sb[:], sel[:, :])
        bias_sb = const.tile([P, 1], F32)
        nc.gpsimd.memset(bias_sb[:], -SHIFT)

        a16_v = a16[:, :].rearrange("p (n k b) -> p n k b", n=NB, k=KC)
        b8_v = b8[:, :].rearrange("p (n k b) -> p n k b", n=NB, k=KC)
        a8_v = a8[:, :].rearrange("p (n k b) -> p n k b", n=NB, k=KC)

        # PE warmup during the DMA ramp: dense zero matmuls flip the HAM
        # clock gate (4096-cycle activity window) to 8/8 before real work
        warm_w = const.tile([P, DM], F16)
        nc.gpsimd.memset(warm_w[:], 0.0)
        warm_m = const.tile([P, NBC], F16)
        nc.gpsimd.memset(warm_m[:], 0.0)
        ps_warm = mm_ps.tile([DM, NBC], F32, tag="ps_s")
        for _ in range(8):
            nc.tensor.matmul(ps_warm[:], warm_w[:], warm_m[:], start=True, stop=True)

        sw_v = sw_d[:, :].rearrange("d (x b) -> d x b", x=2)
        rth_sb = c8t_sb = d8t_sb = None
        pend = None  # deferred tail of the previous block

        def tail_back(ex, prod, cols, halves):
            sw_sb = spool.tile([D, 2, NBC], F32, tag="sw", name="sw_sb")
            for h in halves:
                ps_sums = red_ps.tile([D, NBC], F32, tag="sums", name="ps_sums")
                ps_wsum = red_ps.tile([D, NBC], F32, tag="wsum", name="ps_wsum")
                nc.tensor.matmul(
                    ps_sums[:, h], sel_sb[:], ex[:, h], start=True, stop=True
                )
                nc.tensor.matmul(
                    ps_wsum[:, h], sel_sb[:], prod[:, h], start=True, stop=True
                )
                nc.vector.tensor_copy(sw_sb[:, 0, h], ps_sums[:, h])
                nc.vector.tensor_copy(sw_sb[:, 1, h], ps_wsum[:, h])
                nc.scalar.dma_start(
                    sw_v[:, :, cols.start + h.start : cols.start + h.stop],
                    sw_sb[:, :, h],
                )

        for it in range(NB * repeat):
            blk = it % NB
            cols = slice(blk * NBC, (blk + 1) * NBC)

            a16_sb = fpool.tile([P, KC, NBC], F16, tag="a16")
            b8_sb = fpool.tile([P, KC, NBC], F8, tag="b8")
            a8_sb = fpool.tile([P, KC, NBC], F8, tag="a8")
            if it == 0:
                # block 0 rides one queue in exact need-order: per-chunk a16
                # first (first matmul starts on chunk 0), then rth for the
                # T-mains, then the correction planes and fp8 stationaries
                for k in range(KC):
                    nc.sync.dma_start(a16_sb[:, k, :], a16_v[:, blk, k])
                rth_sb = ld_const("rth_sb", rth, DM, F16, nc.sync)
                nc.sync.dma_start(b8_sb[:], b8_v[:, blk])
                nc.sync.dma_start(a8_sb[:], a8_v[:, blk])
                c8t_sb = ld_const("c8t_sb", c8t, DMP, F8, nc.sync)
                d8t_sb = ld_const("d8t_sb", d8t, DMP, F8, nc.sync)
            else:
                nc.sync.dma_start(a16_sb[:], a16_v[:, blk])
                nc.gpsimd.dma_start(b8_sb[:], b8_v[:, blk])
                nc.scalar.dma_start(a8_sb[:], a8_v[:, blk])

            # all fp16 mains first, then all DoubleRow corrections: one
            # fp16->DR transition per block instead of two (the first DR
            # matmul's 360-col weight load doesn't hide, ~190ns each).
            # Interleaving the two PSUM accumulation groups needs
            # skip_group_check; hardware has_written bits are per element.
            # For the LAST block close ps_s as early as possible instead
            # (per-piece order) so its exp/mul chain overlaps the T MMs.
            last = it == NB * repeat - 1
            ps_s = mm_ps.tile([DM, NBC], F32, tag="ps_s")
            ps_t = mm_ps.tile([DM, NBC], F32, tag="ps_t")
            pieces = ((ps_s, rsh_sb, c8s_sb, d8s_sb), (ps_t, rth_sb, c8t_sb, d8t_sb))

            def emit_mains(ps, r16):
                for k in range(KC):
                    nc.tensor.matmul(
                        ps[:], r16[:, k, :], a16_sb[:, k, :],
                        start=(k == 0), stop=False, skip_group_check=True,
                    )

            def emit_drs(ps, c8, d8):
                for kk in range(0, KC, 2):
                    nc.tensor.matmul(
                        ps[:], c8[:, kk : kk + 2, 0:DM], b8_sb[:, kk : kk + 2, :],
                        start=False, stop=False, perf_mode=DR,
                        skip_group_check=True,
                    )
                for kk in range(0, KC, 2):
                    nc.tensor.matmul(
                        ps[:], d8[:, kk : kk + 2, 0:DM], a8_sb[:, kk : kk + 2, :],
                        start=False, stop=(kk == KC - 2), perf_mode=DR,
                        skip_group_check=True,
                    )

            if last:
                for ps, r16, c8, d8 in pieces:
                    emit_mains(ps, r16)
                    emit_drs(ps, c8, d8)
            else:
                for ps, r16, _, _ in pieces:
                    emit_mains(ps, r16)
                for ps, _, c8, d8 in pieces:
                    emit_drs(ps, c8, d8)

            # previous block's reductions slot in here: their ex/prod are
            # long done, so the PE never stalls on the ACT->DVE chain
            if pend is not None:
                tail_back(*pend)

            halves = (
                (slice(0, NBC // 2), slice(NBC // 2, NBC))
                if last
                else (slice(0, NBC),)
            )
            ex = spool.tile([DM, NBC], F32R, tag="ex")
            prod = spool.tile([DM, NBC], F32R, tag="prod")
            for h in halves:
                nc.scalar.activation(ex[:, h], ps_s[:, h], AF.Exp, bias=bias_sb[0:DM])
                nc.vector.tensor_mul(prod[:, h], ex[:, h], ps_t[:, h])
            pend = (ex, prod, cols, halves)

        # drain the last block's split tail
        tail_back(*pend)

    nc.finalize()
    return nc


def _get_nc():
    if "nc" not in _CACHE:
        _CACHE["nc"] = _build_nc()
    return _CACHE["nc"]


def _host_prep(feature, W_topic, W_domain, memory):
    """Fold memory banks into projections; fp16+fp8 split planes of the
    TAU-scaled row-normalized features, transposed per core."""
    import ml_dtypes

    E4 = ml_dtypes.float8_e4m3

    mem_flat = memory.reshape(D * M, E).astype(np.float64)
    Pm = mem_flat @ W_topic.astype(np.float64)  # [90, 768]
    Qm = mem_flat @ W_domain.astype(np.float64)

    def pack(A):  # [768, cols] -> [128, KC*cols] partition-major
        cols = A.shape[1]
        return np.ascontiguousarray(
            A.reshape(KC, P, cols).transpose(1, 0, 2).reshape(P, KC * cols)
        )

    def r_planes(R):
        C16 = R.astype(np.float32).astype(np.float16)  # [90, 768]
        c8 = (C16.astype(np.float64) * 2.0**-SB).astype(np.float32).astype(E4)
        d8 = ((R - C16.astype(np.float64)) * 2.0**GA).astype(np.float32).astype(E4)
        r16 = pack(np.ascontiguousarray(C16.T))
        padc = np.zeros((IN, DMP), E4)
        padc[:, :DM] = c8.T
        padd = np.zeros((IN, DMP), E4)
        padd[:, :DM] = d8.T
        return r16, pack(padc), pack(padd)

    rsh, c8s, d8s = r_planes(Pm)
    rth, c8t, d8t = r_planes(Qm)
    sel = np.zeros((DM, D), np.float32)
    sel[np.arange(DM), np.arange(DM) // M] = 1.0

    f = np.asarray(feature, dtype=np.float32)
    norm = np.sqrt((f.astype(np.float64) ** 2).sum(axis=1))
    fn = f.astype(np.float64) * (TAU / norm)[:, None]
    a16f = fn.astype(np.float16)                      # [B, 768]
    b8f = ((fn - a16f.astype(np.float64)) * 2.0**SB).astype(np.float32).astype(E4)
    a8f = (fn * 2.0**-GA).astype(np.float32).astype(E4)

    def pack_feat(A):  # [768, BC] -> [P, NB * KC*NBC] block-contiguous
        return np.ascontiguousarray(
            A.reshape(KC, P, NB, NBC)
            .transpose(1, 2, 0, 3)
            .reshape(P, NB * KC * NBC)
        )

    per_core = []
    for c in range(NCORES):
        rows = slice(c * BC, (c + 1) * BC)
        per_core.append(
            {
                "a16": pack_feat(a16f[rows].T),
                "b8": pack_feat(b8f[rows].T),
                "a8": pack_feat(a8f[rows].T),
                "rsh": rsh, "rth": rth,
                "c8s": c8s, "c8t": c8t, "d8s": d8s, "d8t": d8t,
                "sel": sel,
            }
        )
    return per_core


def kernel(feature, category, W_topic, W_domain, memory):
    from concourse.bass_utils import run_bass_kernel_spmd

    in_maps = _host_prep(
        feature, np.asarray(W_topic), np.asarray(W_domain), np.asarray(memory)
    )
    nc = _get_nc()
    res = run_bass_kernel_spmd(nc, in_maps, core_ids=list(range(NCORES)))
    outs = []
    for c in range(NCORES):
        sw = res.results[c]["sw"].astype(np.float64)  # [9, 2*BC]
        datt = sw[:, BC:] / sw[:, :BC]
        e2 = np.exp(datt - datt.max(axis=0, keepdims=True))
        outs.append((e2 / e2.sum(axis=0, keepdims=True)).T)
    full = np.concatenate(outs, axis=0)  # [B, 9]
    return full[:, None, :].astype(np.float32)


# revision 32
# speedup vs baseline: 1.1838x; 1.0322x over previous
"""Trainium2 Bass kernel for nn_MemoryNetwork (scatter_memory).

Computation (reference, per batch row b):
    f = feature / ||feature||                       [B, 768]
    topic = f @ W_topic.T ; dom = f @ W_domain.T    [B, 256]
    att   = softmax_m(TAU * topic . memory[d,m])    [B, 9, 10]
    sep   = sum_m att * memory[d,m]                 [B, 9, 256]
    out   = softmax_d(TAU * sep . dom)              [B, 1, 9]

Reformulation: fold the tiny memory banks into the projections on host:
    S = (TAU f/||f||) @ (mem_flat@W_topic).T   [B, 90]   (softmax_m logits)
    T = (TAU f/||f||) @ (mem_flat@W_domain).T  [B, 90]
    ex    = exp(S - SHIFT)
    sums  = sum_m ex ; wsum = sum_m ex*T       (device outputs, [9, B])
    out   = softmax_d(wsum/sums)               (host, exact fp64)

Device layout: R = [S|T]-projection matrices are the STATIONARY operand
([128 x 90] per 128-row contraction chunk, reused across the whole
batch); the batch streams as the moving operand in 512-column blocks.
This amortizes PE weight loads -- a feature-stationary scheme pays a
~104ns LDWEIGHTS for every ~75ns of streaming.

Precision: the main product runs as fp16 x fp16 (exact products, fp32
PSUM accumulate): f' = a16 + b, R = C16 + D with a16/C16 fp16.
a16@C16 gives the bulk; the two correction terms run as fp8e4m3
DoubleRow matmuls at 1/4 the cycles of an fp16 pair:
    b8@C8   (b scaled 2^6, C16 scaled 2^-6)
    a8@D8   (f' scaled 2^-7, D scaled 2^7)
Dropped b@D term is O(2^-24). The m-group reductions are selector
matmuls with f32r moving (1 cyc/row, tf32-truncated mantissas -- the
dominant remaining error, ~4e-3 rel end to end).

All constants are host-prepacked to [128, k*cols] partition-major so
their DMAs are single contiguous 2D transfers (a rearranging DMA costs
~1.9us of descriptor generation and serializes the queue -- that idled
the PE >3.4us and re-throttled HAM to 1.2 GHz in an earlier version).

Sharding: data-parallel over B across 8 cores (4096 rows each).
"""

import sys

sys.path.insert(0, "/opt/trn_rl_repo")

import numpy as np

B, IN, E, D, M = 32768, 768, 256, 9, 10
NCORES = 8
BC = B // NCORES   # rows per core (4096)
P = 128            # partition tile
KC = IN // P       # contraction chunks (6)
NBC = 512          # batch block columns (one PSUM bank of fp32)
NB = BC // NBC     # blocks per core (8)
DM = D * M         # 90
DMP = 96           # DoubleRow stationary pad (sub-k stride % 16 == 0)
TAU = 32.0
SHIFT = 50.0
SB = 6             # b-plane scale exponent (b*2^SB, C*2^-SB)
GA = 7             # a8/D8 scale exponent (f*2^-GA, D*2^GA)

_CACHE: dict = {}


def _build_nc(repeat=1):
    from contextlib import ExitStack

    import concourse.bacc as bacc
    import concourse.tile as tile
    from concourse import mybir

    F32 = mybir.dt.float32
    F32R = mybir.dt.float32r
    F16 = mybir.dt.float16
    F8 = mybir.dt.float8e4
    AF = mybir.ActivationFunctionType
    DR = mybir.MatmulPerfMode.DoubleRow

    nc = bacc.Bacc(trn_type="TRN2")
    # feature planes host-packed [P, NB * KC*NBC]: one contiguous segment
    # per partition per (block) -> single-descriptor DMAs. The coarse fp8
    # plane a8 = e4m3(a16 * 2^-GA) is derived on-chip (25% less HBM).
    a16 = nc.dram_tensor("a16", [P, NB * KC * NBC], F16, kind="ExternalInput")
    b8 = nc.dram_tensor("b8", [P, NB * KC * NBC], F8, kind="ExternalInput")
    # consts prepacked [128, KC*cols] partition-major
    rsh = nc.dram_tensor("rsh", [P, KC * DM], F16, kind="ExternalInput")
    rth = nc.dram_tensor("rth", [P, KC * DM], F16, kind="ExternalInput")
    c8s = nc.dram_tensor("c8s", [P, KC * DMP], F8, kind="ExternalInput")
    c8t = nc.dram_tensor("c8t", [P, KC * DMP], F8, kind="ExternalInput")
    d8s = nc.dram_tensor("d8s", [P, KC * DMP], F8, kind="ExternalInput")
    d8t = nc.dram_tensor("d8t", [P, KC * DMP], F8, kind="ExternalInput")
    sel = nc.dram_tensor("sel", [DM, D], F32R, kind="ExternalInput")
    sw_d = nc.dram_tensor("sw", [D, 2 * BC], F32, kind="ExternalOutput")

    with tile.TileContext(nc) as tc, ExitStack() as ctx:
        const = ctx.enter_context(tc.tile_pool(name="const", bufs=1))
        fpool = ctx.enter_context(tc.tile_pool(name="fts", bufs=2))
        spool = ctx.enter_context(tc.tile_pool(name="small", bufs=2))
        mm_ps = ctx.enter_context(tc.tile_pool(name="mmps", bufs=2, space="PSUM"))
        red_ps = ctx.enter_context(tc.tile_pool(name="redps", bufs=2, space="PSUM"))

        def ld_const(name, dram, cols, dtype, eng):
            t = const.tile([P, KC, cols], dtype, name=name)
            eng.dma_start(t[:], dram[:, :].rearrange("p (k j) -> p k j", k=KC))
            return t

        # scalar-queue consts lead (rsh feeds the very first matmul);
        # sync-queue consts are issued after block 0's a16 chunks below
        rsh_sb = ld_const("rsh_sb", rsh, DM, F16, nc.scalar)
        c8s_sb = ld_const("c8s_sb", c8s, DMP, F8, nc.scalar)
        d8s_sb = ld_const("d8s_sb", d8s, DMP, F8, nc.scalar)
        sel_sb = const.tile([DM, D], F32R)
        nc.scalar.dma_start(sel_sb[:], sel[:, :])
        bias_sb = const.tile([P, 1], F32)
        nc.gpsimd.memset(bias_sb[:], -SHIFT)

        a16_v = a16[:, :].rearrange("p (n k b) -> p n k b", n=NB, k=KC)
        b8_v = b8[:, :].rearrange("p (n k b) -> p n k b", n=NB, k=KC)

        # PE warmup during the DMA ramp: dense zero matmuls flip the HAM
        # clock gate (4096-cycle activity window) to 8/8 before real work
        warm_w = const.tile([P, DM], F16)
        nc.gpsimd.memset(warm_w[:], 0.0)
        warm_m = const.tile([P, NBC], F16)
        nc.gpsimd.memset(warm_m[:], 0.0)
        ps_warm = mm_ps.tile([DM, NBC], F32, tag="ps_s")
        for _ in range(8):
            nc.tensor.matmul(ps_warm[:], warm_w[:], warm_m[:], start=True, stop=True)

        sw_v = sw_d[:, :].rearrange("d (x b) -> d x b", x=2)
        rth_sb = c8t_sb = d8t_sb = None
        pend = None  # deferred tail of the previous block

        def tail_back(ex, prod, cols, halves):
            sw_sb = spool.tile([D, 2, NBC], F32, tag="sw", name="sw_sb")
            for h in halves:
                ps_sums = red_ps.tile([D, NBC], F32, tag="sums", name="ps_sums")
                ps_wsum = red_ps.tile([D, NBC], F32, tag="wsum", name="ps_wsum")
                nc.tensor.matmul(
                    ps_sums[:, h], sel_sb[:], ex[:, h], start=True, stop=True
                )
                nc.tensor.matmul(
                    ps_wsum[:, h], sel_sb[:], prod[:, h], start=True, stop=True
                )
                nc.vector.tensor_copy(sw_sb[:, 0, h], ps_sums[:, h])
                nc.vector.tensor_copy(sw_sb[:, 1, h], ps_wsum[:, h])
                nc.scalar.dma_start(
                    sw_v[:, :, cols.start + h.start : cols.start + h.stop],
                    sw_sb[:, :, h],
                )

        for it in range(NB * repeat):
            blk = it % NB
            cols = slice(blk * NBC, (blk + 1) * NBC)

            a16_sb = fpool.tile([P, KC, NBC], F16, tag="a16")
            b8_sb = fpool.tile([P, KC, NBC], F8, tag="b8")
            a8_sb = fpool.tile([P, KC, NBC], F8, tag="a8")
            if it == 0:
                # block 0 rides one queue in exact need-order: per-chunk a16
                # first (first matmul starts on chunk 0), then rth for the
                # T-mains, then the correction planes and fp8 stationaries
                for k in range(KC):
                    nc.sync.dma_start(a16_sb[:, k, :], a16_v[:, blk, k])
                rth_sb = ld_const("rth_sb", rth, DM, F16, nc.sync)
                nc.sync.dma_start(b8_sb[:], b8_v[:, blk])
                c8t_sb = ld_const("c8t_sb", c8t, DMP, F8, nc.sync)
                d8t_sb = ld_const("d8t_sb", d8t, DMP, F8, nc.sync)
            else:
                nc.sync.dma_start(a16_sb[:], a16_v[:, blk])
                nc.gpsimd.dma_start(b8_sb[:], b8_v[:, blk])
            # derive the coarse fp8 plane on-chip, split across the two
            # free engines so neither exceeds its per-block budget
            nc.scalar.mul(a8_sb[:, 0 : KC // 2, :], a16_sb[:, 0 : KC // 2, :], 2.0**-GA)
            nc.vector.tensor_scalar_mul(
                a8_sb[:, KC // 2 : KC, :], a16_sb[:, KC // 2 : KC, :], 2.0**-GA
            )

            # all fp16 mains first, then all DoubleRow corrections: one
            # fp16->DR transition per block instead of two (the first DR
            # matmul's 360-col weight load doesn't hide, ~190ns each).
            # Interleaving the two PSUM accumulation groups needs
            # skip_group_check; hardware has_written bits are per element.
            # For the LAST block close ps_s as early as possible instead
            # (per-piece order) so its exp/mul chain overlaps the T MMs.
            last = it == NB * repeat - 1
            ps_s = mm_ps.tile([DM, NBC], F32, tag="ps_s")
            ps_t = mm_ps.tile([DM, NBC], F32, tag="ps_t")
            pieces = ((ps_s, rsh_sb, c8s_sb, d8s_sb), (ps_t, rth_sb, c8t_sb, d8t_sb))

            def emit_mains(ps, r16):
                for k in range(KC):
                    nc.tensor.matmul(
                        ps[:], r16[:, k, :], a16_sb[:, k, :],
                        start=(k == 0), stop=False, skip_group_check=True,
                    )

            def emit_drs(ps, c8, d8):
                for kk in range(0, KC, 2):
                    nc.tensor.matmul(
                        ps[:], c8[:, kk : kk + 2, 0:DM], b8_sb[:, kk : kk + 2, :],
                        start=False, stop=False, perf_mode=DR,
                        skip_group_check=True,
                    )
                for kk in range(0, KC, 2):
                    nc.tensor.matmul(
                        ps[:], d8[:, kk : kk + 2, 0:DM], a8_sb[:, kk : kk + 2, :],
                        start=False, stop=(kk == KC - 2), perf_mode=DR,
                        skip_group_check=True,
                    )

            if last:
                for ps, r16, c8, d8 in pieces:
                    emit_mains(ps, r16)
                    emit_drs(ps, c8, d8)
            else:
                for ps, r16, _, _ in pieces:
                    emit_mains(ps, r16)
                for ps, _, c8, d8 in pieces:
                    emit_drs(ps, c8, d8)

            # previous block's reductions slot in here: their ex/prod are
            # long done, so the PE never stalls on the ACT->DVE chain
            if pend is not None:
                tail_back(*pend)

            halves = (
                (slice(0, NBC // 2), slice(NBC // 2, NBC))
                if last
                else (slice(0, NBC),)
            )
            ex = spool.tile([DM, NBC], F32R, tag="ex")
            prod = spool.tile([DM, NBC], F32R, tag="prod")
            for h in halves:
                nc.scalar.activation(ex[:, h], ps_s[:, h], AF.Exp, bias=bias_sb[0:DM])
                nc.vector.tensor_mul(prod[:, h], ex[:, h], ps_t[:, h])
            pend = (ex, prod, cols, halves)

        # drain the last block's split tail
        tail_back(*pend)

    # Pin Exp and Copy/Identity into one ACT table set so the scalar
    # engine never swaps tables (~2.7us per load) between the exp and
    # the fp8-convert copies.
    mine = {AF.Exp, AF.Ln, AF.Square, AF.Copy, AF.Identity}
    orig_tables = bacc.get_activation_tables

    def _patched(arch):
        return {
            name: (fns if name == "natural_log_exp_and_others" else fns - mine)
            for name, fns in orig_tables(arch).items()
        }

    bacc.get_activation_tables = _patched
    try:
        nc.finalize()
    finally:
        bacc.get_activation_tables = orig_tables
    return nc


def _get_nc():
    if "nc" not in _CACHE:
        _CACHE["nc"] = _build_nc()
    return _CACHE["nc"]


def _host_prep(feature, W_topic, W_domain, memory):
    """Fold memory banks into projections; fp16+fp8 split planes of the
    TAU-scaled row-normalized features, transposed per core."""
    import ml_dtypes

    E4 = ml_dtypes.float8_e4m3

    mem_flat = memory.reshape(D * M, E).astype(np.float64)
    Pm = mem_flat @ W_topic.astype(np.float64)  # [90, 768]
    Qm = mem_flat @ W_domain.astype(np.float64)

    def pack(A):  # [768, cols] -> [128, KC*cols] partition-major
        cols = A.shape[1]
        return np.ascontiguousarray(
            A.reshape(KC, P, cols).transpose(1, 0, 2).reshape(P, KC * cols)
        )

    def r_planes(R):
        C16 = R.astype(np.float32).astype(np.float16)  # [90, 768]
        c8 = (C16.astype(np.float64) * 2.0**-SB).astype(np.float32).astype(E4)
        d8 = ((R - C16.astype(np.float64)) * 2.0**GA).astype(np.float32).astype(E4)
        r16 = pack(np.ascontiguousarray(C16.T))
        padc = np.zeros((IN, DMP), E4)
        padc[:, :DM] = c8.T
        padd = np.zeros((IN, DMP), E4)
        padd[:, :DM] = d8.T
        return r16, pack(padc), pack(padd)

    rsh, c8s, d8s = r_planes(Pm)
    rth, c8t, d8t = r_planes(Qm)
    sel = np.zeros((DM, D), np.float32)
    sel[np.arange(DM), np.arange(DM) // M] = 1.0

    f = np.asarray(feature, dtype=np.float32)
    norm = np.sqrt((f.astype(np.float64) ** 2).sum(axis=1))
    fn = f.astype(np.float64) * (TAU / norm)[:, None]
    a16f = fn.astype(np.float16)                      # [B, 768]
    b8f = ((fn - a16f.astype(np.float64)) * 2.0**SB).astype(np.float32).astype(E4)

    def pack_feat(A):  # [768, BC] -> [P, NB * KC*NBC] block-contiguous
        return np.ascontiguousarray(
            A.reshape(KC, P, NB, NBC)
            .transpose(1, 2, 0, 3)
            .reshape(P, NB * KC * NBC)
        )

    per_core = []
    for c in range(NCORES):
        rows = slice(c * BC, (c + 1) * BC)
        per_core.append(
            {
                "a16": pack_feat(a16f[rows].T),
                "b8": pack_feat(b8f[rows].T),
                "rsh": rsh, "rth": rth,
                "c8s": c8s, "c8t": c8t, "d8s": d8s, "d8t": d8t,
                "sel": sel,
            }
        )
    return per_core


def kernel(feature, category, W_topic, W_domain, memory):
    from concourse.bass_utils import run_bass_kernel_spmd

    in_maps = _host_prep(
        feature, np.asarray(W_topic), np.asarray(W_domain), np.asarray(memory)
    )
    nc = _get_nc()
    res = run_bass_kernel_spmd(nc, in_maps, core_ids=list(range(NCORES)))
    outs = []
    for c in range(NCORES):
        sw = res.results[c]["sw"].astype(np.float64)  # [9, 2*BC]
        datt = sw[:, BC:] / sw[:, :BC]
        e2 = np.exp(datt - datt.max(axis=0, keepdims=True))
        outs.append((e2 / e2.sum(axis=0, keepdims=True)).T)
    full = np.concatenate(outs, axis=0)  # [B, 9]
    return full[:, None, :].astype(np.float32)
